# revision 1
# baseline (speedup 1.0000x reference)
"""Trainium2 Bass kernel for nn_PlanNotesProjection.

Math (per batch b):
  own_f   = ownership[b].astype(f32)             # (K=32, S=4096)
  summed  = own_f @ emb[b]                       # (K, H=2048)
  counts  = clip(own_f.sum(-1), min=1)           # (K,)
  pooled  = summed / counts[:, None]
  proj    = pooled @ W + bias                    # (K, D=1024)
  out[b]  = LayerNorm(proj) * gamma + beta       # eps=1e-5

Structure: h-major. The host pre-swizzles emb so that for each h-tile
(128 columns of H) all 32 S-chunks are one contiguous [128, 16KB] DMA.
Pooling for h-tile h accumulates sumT_h[m, k] = sum_s emb[s, h*128+m] *
own[k, s] over the 32 S-chunks into a dedicated PSUM bank; as soon as an
h-tile finishes, its two projection matmuls (contraction over H lands on
partitions — no transpose) accumulate into the proj PSUM banks while the
next h-tile's DMA/pooling proceeds. This hides nearly all projection
work behind the emb streaming, leaving only the last h-tile's pooling +
projection + LayerNorm epilogue as serial tail. The 1/counts scaling
commutes past the projection matmul, so it is applied to proj instead.

Sharding: data-parallel over B across 8 cores (one batch per core);
W/b/gamma/beta replicated. Host swizzles make every device DMA fully
contiguous per partition:
  embP[p, (h*SC + c)*128 + j] = emb[c*128+p, h*128+j]
  ownP[p, c*K + k]            = own[k, c*128+p]
  wP[p, h*D + d]              = W[h*128+p, d]
"""

import sys
from contextlib import ExitStack

import numpy as np

sys.path.insert(0, "/opt/trn_rl_repo")

B, K, S, H, D = 8, 32, 4096, 2048, 1024
LN_EPS = 1e-5
P = 128
SC = S // P    # 32 contraction chunks (S on partitions)
HC = H // P    # 16 h-tiles
DJ = D // 512  # 2 psum column tiles for projection

TRACE = False
LAST_RESULT = None
_NC = None


def _prep_emb(emb_b):
    # (S, H) f32 -> (P, HC*SC*128) with embP[p, (h*SC+c)*128+j] = emb[c*128+p, h*128+j]
    return np.ascontiguousarray(
        emb_b.reshape(SC, P, HC, P).transpose(1, 2, 0, 3).reshape(P, HC * SC * P))


def _prep_own(own_b):
    # (K, S) bool -> (P, SC*K) f32 with ownP[p, c*K+k] = own[k, c*128+p]
    return np.ascontiguousarray(
        own_b.T.astype(np.float32).reshape(SC, P, K).transpose(1, 0, 2).reshape(P, SC * K))


def _prep_w(wmat):
    # (H, D) f32 -> (P, HC*D) with wP[p, h*D+d] = W[h*128+p, d]
    return np.ascontiguousarray(
        wmat.reshape(HC, P, D).transpose(1, 0, 2).reshape(P, HC * D))


def _build_nc(repeats=1):
    # repeats>1 unrolls the whole compute body (including DMAs) multiple
    # times in one NEFF; used by test.py to measure marginal per-iteration
    # HW time, cancelling host dispatch overhead. Grading uses repeats=1.
    import concourse.bass as bass
    import concourse.tile as tile
    from concourse import mybir
    from concourse.bacc import Bacc

    FP32 = mybir.dt.float32

    # Bacc (not plain Bass): its finalize() runs the legalization passes
    # (move_matmul_waits_to_ldweights, generate_event_semaphores) that split
    # multi-semaphore waits — TRN2 TPB instructions carry at most one.
    nc = Bacc("TRN2", target_bir_lowering=False)
    embP = nc.declare_dram_parameter("embP", [P, HC * SC * P], FP32, False)
    ownP = nc.declare_dram_parameter("ownP", [P, SC * K], FP32, False)
    wP = nc.declare_dram_parameter("wP", [P, HC * D], FP32, False)
    bvec = nc.declare_dram_parameter("bvec", [D], FP32, False)
    gamma = nc.declare_dram_parameter("gamma", [D], FP32, False)
    beta = nc.declare_dram_parameter("beta", [D], FP32, False)
    out = nc.declare_dram_parameter("out", [K, D], FP32, True)

    with ExitStack() as ctx:
        tc = ctx.enter_context(tile.TileContext(nc))

        own_pool = ctx.enter_context(tc.tile_pool(name="own", bufs=1))
        w_pool = ctx.enter_context(tc.tile_pool(name="w", bufs=1))
        emb_pool = ctx.enter_context(tc.tile_pool(name="emb", bufs=7))
        ones_pool = ctx.enter_context(tc.tile_pool(name="ones", bufs=1))
        eps_pool = ctx.enter_context(tc.tile_pool(name="eps", bufs=1))
        cnt_pool = ctx.enter_context(tc.tile_pool(name="cnt", bufs=1))
        st_pool = ctx.enter_context(tc.tile_pool(name="st", bufs=2))
        bc_pool = ctx.enter_context(tc.tile_pool(name="bc", bufs=1))
        x_pool = ctx.enter_context(tc.tile_pool(name="x", bufs=1))
        stats_pool = ctx.enter_context(tc.tile_pool(name="stats", bufs=1))
        mv_pool = ctx.enter_context(tc.tile_pool(name="mv", bufs=1))

        # Every PSUM allocation is rounded up to whole banks (bump_psum), so
        # each sumT ping-pong buffer owns a full bank: a start=True matmul's
        # whole-bank zero touches only its own accumulation group.
        psum_sum = ctx.enter_context(tc.tile_pool(name="psum_sum", bufs=2, space="PSUM"))
        psum_proj = ctx.enter_context(tc.tile_pool(name="psum_proj", bufs=1, space="PSUM"))
        psum_cnt = ctx.enter_context(tc.tile_pool(name="psum_cnt", bufs=1, space="PSUM"))

        def body():
            own_sb = own_pool.tile([P, SC, K], FP32)
            nc.sync.dma_start(own_sb[:], ownP[:, :])

            w_sb = w_pool.tile([P, HC, D], FP32)
            nc.sync.dma_start(w_sb[:, 0, :], wP[:, 0:D])

            ones = ones_pool.tile([P, 1], FP32)
            nc.vector.memset(ones[:], 1.0)
            eps = eps_pool.tile([K, 1], FP32)
            nc.vector.memset(eps[:], LN_EPS)

            def bcast(vec):
                t = bc_pool.tile([K, D], FP32, name=f"bc_{vec.name}")
                ap = vec[:]
                bc_ap = bass.AP(tensor=ap.tensor, offset=ap.offset, ap=[[0, K]] + list(ap.ap))
                nc.gpsimd.dma_start(out=t[:], in_=bc_ap)
                return t

            bias_bc = bcast(bvec)
            gam_bc = bcast(gamma)
            bet_bc = bcast(beta)

            # counts[k] = sum_s own[k, s]
            cnt_ps = psum_cnt.tile([K, 1], FP32)
            for c in range(SC):
                nc.tensor.matmul(cnt_ps[:], own_sb[:, c, :], ones[:],
                                 start=(c == 0), stop=(c == SC - 1))
            cnt_sb = cnt_pool.tile([K, 1], FP32)
            nc.vector.tensor_scalar_max(out=cnt_sb[:], in0=cnt_ps[:], scalar1=1.0)
            inv_sb = cnt_pool.tile([K, 1], FP32)
            nc.vector.reciprocal(out=inv_sb[:], in_=cnt_sb[:])

            proj_ps = [psum_proj.tile([K, 512], FP32, name=f"proj_ps{jj}") for jj in range(DJ)]

            HB = SC // 2  # half an h-tile's chunks per DMA
            for h in range(HC):
                # Two half-loads per h-tile keep 8 DMAs (4 h-tiles) in flight
                # across the 8 HWDGE lanes instead of 3 monolithic ones.
                base = h * SC * P
                etA = emb_pool.tile([P, HB, P], FP32)
                nc.sync.dma_start(etA[:], embP[:, base:base + HB * P])
                etB = emb_pool.tile([P, HB, P], FP32)
                nc.sync.dma_start(etB[:], embP[:, base + HB * P:base + SC * P])
                if h + 1 < HC:
                    nc.sync.dma_start(w_sb[:, h + 1, :], wP[:, (h + 1) * D:(h + 2) * D])

                # Padded to 512 cols = 2KB = one full bank, so each ping-pong
                # buf owns its bank and start=True can't touch a neighbour.
                st_ps = psum_sum.tile([P, 512], FP32)
                for c in range(SC):
                    et = etA[:, c, :] if c < HB else etB[:, c - HB, :]
                    nc.tensor.matmul(st_ps[:, 0:K], et, own_sb[:, c, :],
                                     start=(c == 0), stop=(c == SC - 1))
                st_sb = st_pool.tile([P, K], FP32)
                nc.scalar.copy(out=st_sb[:], in_=st_ps[:, 0:K])
                for jj in range(DJ):
                    nc.tensor.matmul(proj_ps[jj][:], st_sb[:], w_sb[:, h, jj * 512:(jj + 1) * 512],
                                     start=(h == 0), stop=(h == HC - 1))

            # --- epilogue: x = proj_raw/counts + bias; LayerNorm; *gamma + beta ---
            x = x_pool.tile([K, D], FP32)
            for jj in range(DJ):
                nc.vector.tensor_scalar_mul(
                    out=x[:, jj * 512:(jj + 1) * 512], in0=proj_ps[jj][:], scalar1=inv_sb[:],
                )
            nc.vector.tensor_add(out=x[:], in0=x[:], in1=bias_bc[:])

            stats = stats_pool.tile([K, DJ, nc.vector.BN_STATS_DIM], FP32)
            for g in range(DJ):
                nc.vector.bn_stats(out=stats[:, g, :], in_=x[:, g * 512:(g + 1) * 512])
            mv = mv_pool.tile([K, nc.vector.BN_AGGR_DIM], FP32)
            nc.vector.bn_aggr(out=mv[:], in_=stats[:])
            nc.scalar.activation(
                out=mv[:, 1:2], in_=mv[:, 1:2],
                func=mybir.ActivationFunctionType.Sqrt, bias=eps[:], scale=1.0, alpha=0.0,
            )
            nc.vector.reciprocal(out=mv[:, 1:2], in_=mv[:, 1:2])
            normed = x_pool.tile([K, D], FP32)
            nc.vector.tensor_scalar(
                out=normed[:], in0=x[:], scalar1=mv[:, 0:1], scalar2=mv[:, 1:2],
                op0=mybir.AluOpType.subtract, op1=mybir.AluOpType.mult,
            )
            nc.vector.tensor_mul(out=normed[:], in0=normed[:], in1=gam_bc[:])
            outt = x_pool.tile([K, D], FP32)
            nc.vector.tensor_add(out=outt[:], in0=normed[:], in1=bet_bc[:])
            nc.sync.dma_start(out[:, :], outt[:])

        for _ in range(repeats):
            body()

    nc.finalize()
    return nc


def kernel(**inputs: np.ndarray) -> np.ndarray:
    global _NC, LAST_RESULT
    from concourse.bass_utils import run_bass_kernel_spmd

    emb = np.asarray(inputs["plan_embeddings"], dtype=np.float32)
    own = np.asarray(inputs["ownership"])
    wmat = np.ascontiguousarray(np.asarray(inputs["W"], dtype=np.float32))
    bv = np.ascontiguousarray(np.asarray(inputs["b"], dtype=np.float32))
    ga = np.ascontiguousarray(np.asarray(inputs["gamma"], dtype=np.float32))
    be = np.ascontiguousarray(np.asarray(inputs["beta"], dtype=np.float32))

    if _NC is None:
        _NC = _build_nc()

    wP = _prep_w(wmat)
    in_maps = []
    for i in range(B):
        in_maps.append({
            "embP": _prep_emb(emb[i]),
            "ownP": _prep_own(own[i]),
            "wP": wP,
            "bvec": bv,
            "gamma": ga,
            "beta": be,
        })
    res = run_bass_kernel_spmd(_NC, in_maps, core_ids=list(range(B)), trace=TRACE)
    LAST_RESULT = res
    return np.stack([np.asarray(res.results[i]["out"]) for i in range(B)], axis=0).astype(np.float32)

